# revision 23
# baseline (speedup 1.0000x reference)
"""MoE dispatcher kernel for Trainium2 (8 NeuronCores, expert-parallel).

Contract: kernel(**inputs) takes FULL inputs and returns the FULL output.

Strategy (expert-parallel, matches the sharding hint):
  - host: softmax(gate_logits) -> top-2 -> combine weights per (token, expert)
  - host "all-to-all dispatch": for expert e, gather its routed tokens,
    pre-scale rows by the combine weight (w * (x @ W) == (w*x) @ W), pad to a
    common capacity C, transpose to [D, C] so the device streams tokens along
    the free dim.  One expert per core.
  - device (per core): Y^T[D,C] = W[e]^T @ X^T via PE array, tiled
    [128 x <=480] PSUM accumulation over K=D.  (512-wide moving tiles
    measure ~259ns/MM instead of N/2.4+2.5; narrower tiles hit roofline.)
  - host "all-to-all combine": scatter-add each expert's Y rows back to the
    token axis (plain add; weights were folded into x).

DRAM layouts are host-permuted so every DMA is fully contiguous per
partition:
  w   [P, MT*KT*128]       w[p, (mi*KT+k)*128+dd] = W[e][k*128+p, mi*128+dd]
  xt  [NT, P, KT*NSPLIT]   xt[j, p, k*nsz_j + n] = X^T[k*128 + p, n0_j + n]
  yt  [NT, P, MT*NSPLIT]   yt[j, p, mi*nsz_j + n] = Y^T[mi*128 + p, n0_j + n]

DMA plan (HBM writes pay ~2us receipt per transfer -> batch outputs; two
HWDGE queues in exact consumption order):
  scalar: X0(k0-3), X1, X2, out_j0, out_j1
  sync:   W_mi0, X0(k4-7), W_mi1..W_mi7, out_j2
Warmup matmuls cover the DMA lead-in so the HAM clock gate is at 8/8 when
the real matmuls start.
"""

import os

import numpy as np

N_CORES = 8
P = 128
NSPLIT = 480  # moving-tile width (<=512 PSUM bank; 512 itself is slow)

# matmul input dtype and device output dtype
MM_DT = os.environ.get("BASS_MOE_DT", "bfloat16")
OUT_DT = os.environ.get("BASS_MOE_OUT_DT", "bfloat16")
WARMUP_MM = int(os.environ.get("BASS_MOE_WARMUP", "8"))

_prog_cache: dict = {}


def _np_dt(name):
    if name == "bfloat16":
        import ml_dtypes

        return ml_dtypes.bfloat16
    return np.float32


def _n_tiles(C):
    """Split C into NSPLIT-wide tiles plus a remainder (last smallest)."""
    out = []
    n0 = 0
    while C - n0 > NSPLIT:
        out.append((n0, NSPLIT))
        n0 += NSPLIT
    out.append((n0, C - n0))
    return out


def _build_program(D: int, C: int, mm_dt_name: str, out_dt_name: str):
    import concourse.bacc as bacc
    import concourse.mybir as mybir
    import concourse.tile as tile

    mm_dt = getattr(mybir.dt, mm_dt_name)
    out_dt = getattr(mybir.dt, out_dt_name)
    KT = D // P  # k tiles (contraction)
    MT = D // P  # m tiles (output features)
    KH = KT // 2
    n_tiles = _n_tiles(C)
    NT = len(n_tiles)

    nc = bacc.Bacc(None, target_bir_lowering=False)
    xt = nc.declare_dram_parameter("xt", [NT, P, KT * NSPLIT], mm_dt, isOutput=False)
    w = nc.declare_dram_parameter("w", [P, MT * KT * P], mm_dt, isOutput=False)
    yt = nc.declare_dram_parameter("yt", [NT, P, MT * NSPLIT], out_dt, isOutput=True)

    with tile.TileContext(nc) as tc:
        with (
            tc.tile_pool(name="wpool", bufs=MT) as wpool,
            tc.tile_pool(name="xpool", bufs=NT + 2) as xpool,
            tc.tile_pool(name="psum", bufs=3, space="PSUM") as psum_pool,
            tc.tile_pool(name="opool", bufs=3) as opool,
            tc.tile_pool(name="warm", bufs=1) as warmpool,
        ):
            if WARMUP_MM:
                # Keep the PE busy during the DMA lead-in so the HAM clock
                # gate is at 8/8 when the real matmuls start.
                wt = warmpool.tile([P, NSPLIT], mybir.dt.bfloat16, tag="warm_w")
                nc.vector.memset(wt[:], 0.0)
                for i in range(WARMUP_MM):
                    wp = psum_pool.tile([P, NSPLIT], mybir.dt.float32, tag="ps")
                    nc.tensor.matmul(
                        wp[:], lhsT=wt[:, :P], rhs=wt[:], start=True, stop=True
                    )

            # Input DMAs.  The j0 X halves + W chunks ride the two HWDGE
            # queues immediately, interleaved in need-order.  All later X
            # tiles are dependency-delayed (dummy write -> WAW dep on an
            # early eviction) so their transfers cannot steal HBM bandwidth
            # from the critical W stream; the scheduler must obey real deps.
            #   scalar: X0a, W2, W4, W5, W6, W7, [delayed X], outs
            #   sync:   W0, X0b, W1, W3, [delayed X], outs
            #   gpsimd: [delayed X]
            n0_first, nsz_first = n_tiles[0]
            # parts[j] = list of (tile, k0, kn); delayed[(dj,dmi)] = DMAs to
            # issue right after eviction (dj,dmi).
            parts = [[] for _ in n_tiles]
            delayed = {}

            def x_part(j, k0, kn):
                nsz = n_tiles[j][1]
                t = xpool.tile([P, kn, nsz], mm_dt, tag="x_sb")
                parts[j].append((t, k0, kn))
                return t

            def x_dma(eng, j, t, k0, kn):
                nsz = n_tiles[j][1]
                eng.dma_start(
                    t[:].rearrange("p k n -> p (k n)"),
                    xt[j, :, k0 * nsz : (k0 + kn) * nsz],
                )

            x0a = x_part(0, 0, KH)
            nc.scalar.dma_start(
                x0a[:].rearrange("p k n -> p (k n)"), xt[0, :, : KH * nsz_first]
            )

            def w_dma(eng, mi):
                tw = wpool.tile([P, KT, P], mm_dt, tag="w_sb")
                eng.dma_start(
                    tw[:].rearrange("p k d -> p (k d)"),
                    w[:, mi * KT * P : (mi + 1) * KT * P],
                )
                return tw

            w_sb = [None] * MT
            w_sb[0] = w_dma(nc.sync, 0)
            x0b = x_part(0, KH, KT - KH)
            nc.sync.dma_start(
                x0b[:].rearrange("p k n -> p (k n)"),
                xt[0, :, KH * nsz_first : KT * nsz_first],
            )
            # Alternate W chunks across the queues so consecutive mi-chunks
            # arrive ~1.25us apart vs the 1.63us/group compute cadence.
            for mi in range(1, MT):
                w_sb[mi] = w_dma(nc.scalar if mi % 2 else nc.sync, mi)

            for j in range(1, NT):
                if j == 1 and NT > 2:
                    ta = x_part(1, 0, KH)
                    tb = x_part(1, KH, KT - KH)
                    delayed[(0, 0)] = [
                        (nc.gpsimd, 1, ta, 0, KH),
                        (nc.scalar, 1, tb, KH, KT - KH),
                    ]
                else:
                    t = x_part(j, 0, KT)
                    dep = (0, 2) if j == 1 else (0, 2 * j - 2)
                    delayed.setdefault(dep, []).append((nc.sync, j, t, 0, KT))

            def rhs(j, k, nsz):
                for t, k0, kn in parts[j]:
                    if k0 <= k < k0 + kn:
                        return t[:, k - k0, :nsz]
                raise AssertionError

            # Fused compute: per (j, mi) accumulate over k; batch each j's
            # evictions into one SBUF buffer and DMA it out in one shot.
            # The last (smallest) tile's output is split across both queues
            # so the HBM-write receipts overlap the final matmuls.
            for j, (n0, nsz) in enumerate(n_tiles):
                last = j == NT - 1
                mh = MT - 2 if last else MT
                ot = opool.tile([P, mh, nsz], out_dt, tag="ot")
                ot2 = None
                if last:
                    ot2 = opool.tile([P, 2, nsz], out_dt, tag="ot2")
                for mi in range(MT):
                    ps = psum_pool.tile([P, NSPLIT], mybir.dt.float32, tag="ps")
                    for k in range(KT):
                        nc.tensor.matmul(
                            ps[:, :nsz],
                            lhsT=w_sb[mi][:, k, :],
                            rhs=rhs(j, k, nsz),
                            start=(k == 0),
                            stop=(k == KT - 1),
                        )
                    dst = ot[:, mi, :] if mi < mh else ot2[:, mi - mh, :]
                    nc.vector.tensor_copy(dst, ps[:, :nsz])
                    for eng, jx, t, k0, kn in delayed.pop((j, mi), []):
                        # Dummy write -> WAW dep: this X DMA cannot start
                        # before the (j, mi) eviction just above.
                        nc.vector.tensor_copy(t[:, 0, 0:1], dst[:, 0:1])
                        x_dma(eng, jx, t, k0, kn)
                    if last and mi == mh - 1:
                        nc.sync.dma_start(
                            yt[j, :, : mh * nsz],
                            ot[:].rearrange("p m n -> p (m n)"),
                        )
                if last:
                    nc.scalar.dma_start(
                        yt[j, :, mh * nsz : MT * nsz],
                        ot2[:].rearrange("p m n -> p (m n)"),
                    )
                else:
                    nc.scalar.dma_start(
                        yt[j, :, : MT * nsz],
                        ot[:].rearrange("p m n -> p (m n)"),
                    )
    nc.compile()
    return nc


def kernel(hidden: np.ndarray, gate_logits: np.ndarray, W: np.ndarray) -> np.ndarray:
    from concourse.bass_utils import run_bass_kernel_spmd

    hidden = np.asarray(hidden)
    gate_logits = np.asarray(gate_logits)
    W = np.asarray(W)
    B, S, D = hidden.shape
    T, E = gate_logits.shape
    assert E == N_CORES
    x = np.ascontiguousarray(hidden.reshape(T, D).astype(np.float32))

    # --- routing on host (fp32, matches reference softmax/top-2) ---
    g = gate_logits.astype(np.float32)
    m = g.max(axis=-1, keepdims=True)
    p = np.exp(g - m)
    p /= p.sum(axis=-1, keepdims=True)
    top2 = np.argpartition(-p, 1, axis=-1)[:, :2]

    routed = [np.nonzero((top2 == e).any(axis=1))[0] for e in range(E)]
    counts = np.array([len(r) for r in routed])
    C = max(256, -(-int(counts.max()) // 16) * 16)  # capacity, multiple of 16

    mm_np = _np_dt(MM_DT)
    KT = D // P
    MT = D // P
    n_tiles = _n_tiles(C)
    NT = len(n_tiles)

    in_maps = []
    for e in range(E):
        idx = routed[e]
        scale = p[idx, e].astype(np.float32)
        xe = x[idx] * scale[:, None]  # [cnt, D]
        xt_full = np.zeros((D, C), dtype=mm_np)
        xt_full[:, : len(idx)] = xe.T.astype(mm_np)
        # [D, C] -> [KT, P, C] -> per-n-tile [NT, P, KT, nsz] contiguous
        xk = xt_full.reshape(KT, P, C)
        xt_dram = np.zeros((NT, P, KT * NSPLIT), dtype=mm_np)
        for j, (n0, nsz) in enumerate(n_tiles):
            blk = xk[:, :, n0 : n0 + nsz].transpose(1, 0, 2)  # [P, KT, nsz]
            xt_dram[j, :, : KT * nsz] = blk.reshape(P, KT * nsz)
        # w[p, (mi*KT+k)*128+dd] = W[e][k*128+p, mi*128+dd]
        w_full = (
            W[e]
            .astype(mm_np)
            .reshape(KT, P, MT, P)
            .transpose(1, 2, 0, 3)
            .reshape(P, MT * KT * P)
        )
        in_maps.append(
            {
                "xt": np.ascontiguousarray(xt_dram),
                "w": np.ascontiguousarray(w_full),
            }
        )

    key = (D, C, MM_DT, OUT_DT, WARMUP_MM)
    if key not in _prog_cache:
        _prog_cache[key] = _build_program(D, C, MM_DT, OUT_DT)
    nc = _prog_cache[key]

    res = run_bass_kernel_spmd(nc, in_maps, core_ids=list(range(N_CORES)))

    # --- combine on host ---
    out = np.zeros((T, D), dtype=np.float32)
    for e in range(E):
        idx = routed[e]
        yt_res = res.results[e]["yt"]  # [NT, P, MT*NSPLIT]
        ye_t = np.empty((MT, P, C), dtype=np.float32)
        for j, (n0, nsz) in enumerate(n_tiles):
            arr = yt_res[j, :, : MT * nsz].reshape(P, MT, nsz).astype(np.float32)
            ye_t[:, :, n0 : n0 + nsz] = arr.transpose(1, 0, 2)
        ye_t = ye_t.reshape(D, C)
        out[idx] += ye_t[:, : len(idx)].T
    return out.reshape(B, S, D)
